# revision 1
# baseline (speedup 1.0000x reference)
"""Causal attention kernel for Trainium2 (Bass/Tile), 8 NeuronCores. v2.

Problem: B=4, S=4096, D=64 fp32 causal softmax attention.

Sharding: data-parallel over batch (4 batches x 2 cores each); within a
batch the S axis of Q is split causally-balanced: core parity a takes
q-supers (512 rows) at q0 = 512*(2i+a), i=0..3.  Every core runs the SAME
static program over k-context "slots" [1024, 2048, 3072, 4096]; cores whose
causal context is shorter than the slot get host-side zero-padded K/V rows
(zero V rows, incl. the fused ones-column, contribute nothing to numerator
or denominator, so no masking is needed for the padding).

v2 changes vs v1:
  * exp is SPLIT between ScalarE (ACT exp) and VectorE via a custom fused
    8-stage DVE op (EXP2_SCHR_ANT): the host pre-scales Q^T by
    log2e*2^23/sqrt(D) so PSUM scores arrive as t*2^23 (t = logit*log2e).
    The DVE op computes int32((t+191+c+h(frac t))*2^23) in one pass:
    anchor-add pins the fp32 exponent, BITWISE_AND/OR extract the floored
    fraction exactly, and a minimax quadratic h(f)~2^f-1-f corrects the
    linear-mantissa error (max rel err ~2.5e-3).  The int32 bit pattern
    IS the fp32 exp value scaled by 2^64; the host pre-scales the V blocks
    assigned to the DVE path by 2^-64 so PV contributions match ScalarE's.
    ScalarE path: ACT exp with scale=ln2/2^23 on the same pre-scaled scores.
  * no on-chip transpose/normalize tail: O^T (with the fused row-sums in
    row 64) is DMA'd straight from PSUM to DRAM; the host transposes and
    divides during unshard.

Math layout (per 512-row q-super, context slot L): scores^T[k,q] = K @ Q^T
per 128-row k-block (contraction D=64 on partitions; even/odd k-blocks on
partition halves 0-63/64-127 so adjacent matmuls hit disjoint PE row-groups
and overlap).  P^T tiles feed O^T[d,q] += matmul with V_aug [128, 65] as
stationary (col 64 = ones -> row sums in O^T row 64).  Diagonal k-blocks are
free-dim-trimmed, ScalarE-exp'd and triangle-masked with a host triu mask.

KERNEL_REPS (build-time env, default 1): wraps the body in a hardware
For_i loop for benchmarking; the shipped/default program has no loop.
"""

import os
import numpy as np

B, S, D = 4, 4096, 64
NCORES = 8
SUPER = 512  # q rows per super-block
SLOTS = [1024, 2048, 3072, 4096]  # k-context slot per super index
NSUP = 4  # supers per core
QLOC = NSUP * SUPER  # 2048 local q rows
GTOT = sum(SLOTS) // 128  # 80 k-blocks total per core
VW = D + 1  # v row width with ones column

LOG2E = 1.4426950408889634
QSCALE = LOG2E * (1 << 7) / np.sqrt(D)  # host premultiplies Q^T by this
ASCALE = float(np.log(2.0) / (1 << 7))  # ScalarE ACT scale immediate
# minimax quadratic for h(f) = 2^f - 1 - f on [0,1]
HA, HB, HC = 0.34399492, -0.34894542, 0.00247407
C0V = float(np.float32((191 + HC) * (1 << 7)))  # anchor + poly const + exp bias
C1V = float(np.float32(HA * 2.0**21))
C2V = float(np.float32(HB * 2.0**14))
VSCALE = 2.0**-64  # host scale on V blocks consumed via the DVE path

GRP = int(os.environ.get("KERNEL_GROUP", "3"))
# target DVE-path full-block columns per core (engine balance knob)
DVE_COLS = int(os.environ.get("KERNEL_DVE_COLS", "15872"))
USE_DVE = os.environ.get("KERNEL_DVE", "1") == "1"

_CACHE = {}
_G0S = [sum(SLOTS[:i]) // 128 for i in range(NSUP)]


def _schedule():
    """Assign each full-block group to 'dve' or 'act', spread evenly."""
    fulls = []
    for i in range(NSUP):
        nb = SLOTS[i] // 128
        for j in range(4, nb, GRP):
            nt = min(GRP, nb - j)
            fulls.append((i, j, nt))
    sched = {}
    if not USE_DVE:
        return {(i, j): "act" for i, j, _ in fulls}
    total = sum(nt * 512 for _, _, nt in fulls)
    frac = min(1.0, DVE_COLS / total)
    dve = run = 0.0
    for i, j, nt in fulls:
        cols = nt * 512
        run += cols
        if dve + cols <= frac * run + 1e-9:
            sched[(i, j)] = "dve"
            dve += cols
        else:
            sched[(i, j)] = "act"
    return sched


def _dve_blocks(sched):
    """Set of global block indices (0..GTOT-1) consumed via the DVE path."""
    out = set()
    for (i, j), eng in sched.items():
        if eng == "dve":
            nb = SLOTS[i] // 128
            for t in range(min(GRP, nb - j)):
                out.add(_G0S[i] + j + t)
    return out


def _register_exp2():
    """Register the custom fused DVE exp op (idempotent)."""
    from concourse import dve_ops
    from concourse.dve_spec import (
        Spec, Src0, C0, C1, C2, C3, One, Bin, AluOp, lower,
        _spill_c3_to_src1,
    )
    from concourse.dve_uop import DveOpSpec

    name = "EXP2_SCHR_ANT"
    for op in dve_ops.OPS:
        if op.name == name:
            return op

    # the 0xFFFF frac mask rides C3 (latched from in1 at element 0 — the
    # streaming [P,1] Src1 broadcast hangs the DVE on this runtime)
    a = Src0 + C0
    fb = Bin(AluOp.BITWISE_AND, a, C3)
    fo = Bin(AluOp.BITWISE_OR, fb, One)
    fm = fo - One
    body = _spill_c3_to_src1(a + (fm * C1 + C2) * fm)

    def ref(in0, in1, s0, s1, imm2):
        av = (np.asarray(in0, np.float32) + np.float32(s0)).astype(np.float32)
        m = np.broadcast_to(np.asarray(in1, np.float32), av.shape)
        fb_ = av.view(np.uint32) & m.view(np.uint32)
        fo_ = fb_ | np.float32(1.0).view(np.uint32)
        fm_ = (fo_.view(np.float32) - np.float32(1.0)).astype(np.float32)
        p3 = ((fm_ * np.float32(s1) + np.float32(imm2)) * fm_).astype(np.float32)
        return (av + p3).astype(np.float32)

    spec = Spec(body=body, reference=ref)
    row = dve_ops._CUSTOM_DVE_ROW_BASE + len(dve_ops.OPS)
    shas = {
        ver: DveOpSpec(name=name, opcode=row, uops=lower(spec, ver=ver),
                       rd1_en=True).sha(ver)
        for ver in ("v3", "v4")
    }
    op = dve_ops.DveOp(name, spec, subdim=False, uops_sha=shas)
    dve_ops.OPS.append(op)
    dve_ops._SUB_OPCODE_FOR_NAME[name] = row
    dve_ops.CUSTOM_DVE_SPECS[name] = spec
    return op


def _build_nc(reps=None):
    import concourse.bass as bass
    import concourse.tile as tile
    import concourse.mybir as mybir
    from concourse import bacc

    if reps is None:
        reps = int(os.environ.get("KERNEL_REPS", "1"))
    f32 = mybir.dt.float32
    i16 = mybir.dt.int16
    bf16 = mybir.dt.bfloat16
    nc = bacc.Bacc("TRN2", target_bir_lowering=False, debug=False, num_devices=NCORES)

    # float32r: PE fast-mode fp32 (TF32-like rounding), 1 cycle/row.
    use_fr = os.environ.get("KERNEL_F32R", "1") == "1"
    fdt = mybir.dt.float32r if use_fr else f32
    fr = lambda ap: ap

    exp2_op = _register_exp2() if USE_DVE else None
    sched = _schedule()

    qt_d = nc.dram_tensor("qt", [128, QLOC], fdt, kind="ExternalInput").ap()
    kt_d = nc.dram_tensor("kt", [128, GTOT // 2 * 128], fdt, kind="ExternalInput").ap()
    vp_d = nc.dram_tensor("vp", [128, GTOT * VW], bf16, kind="ExternalInput").ap()
    tri_d = nc.dram_tensor("tri", [128, 128], bf16, kind="ExternalInput").ap()
    msk_d = nc.dram_tensor("msk", [128, 1], f32, kind="ExternalInput").ap()
    o_d = nc.dram_tensor("o", [VW, QLOC], f32, kind="ExternalOutput").ap()

    Exp = mybir.ActivationFunctionType.Exp
    TRIENG = os.environ.get("KERNEL_TRIENG", "vector")
    ILV = os.environ.get("KERNEL_ILV", "1") == "1"

    PSW = 512 * GRP
    with tile.TileContext(nc) as tc:
        with (
            tc.tile_pool(name="inp", bufs=int(os.environ.get("KERNEL_INPBUFS", "2"))) as inp,
            tc.tile_pool(name="pexp_pool", bufs=4) as pe_pool,
            tc.tile_pool(name="pexpi_pool", bufs=4) as pi_pool,
            tc.tile_pool(name="ocp_pool", bufs=2) as ocp_pool,
            tc.tile_pool(name="ps_pool", bufs=max(2, (8 - 2) // GRP), space="PSUM") as ps_pool,
            tc.tile_pool(name="oacc_pool", bufs=2, space="PSUM") as oacc_pool,
        ):
            tri_s = inp.tile([128, 128], bf16)
            msk_s = inp.tile([128, 1], f32)

            part = os.environ.get("KERNEL_PART", "all")  # all|dma|compute
            held = {}

            def loads():
                kt_s = inp.tile([128, GTOT // 2 * 128], fdt, tag="kt_s", name="kt_s")
                vp_s = inp.tile([128, GTOT * VW], bf16, tag="vp_s", name="vp_s")
                qt_s = inp.tile([128, QLOC], fdt, tag="qt_s", name="qt_s")
                # load order tuned for the ~650ns-per-issue DMA queues;
                # issue in parallel on both HWDGE queues (SP + Activation):
                # super-0 critical chunks first, then tri/msk, then the rest
                nc.sync.dma_start(kt_s[:, 0:256], kt_d[:, 0:256])
                nc.scalar.dma_start(qt_s[:, 0:SUPER], qt_d[:, 0:SUPER])
                nc.sync.dma_start(vp_s[:, 0 : 4 * VW], vp_d[:, 0 : 4 * VW])
                nc.scalar.dma_start(tri_s[:], tri_d)
                nc.sync.dma_start(kt_s[:, 256:512], kt_d[:, 256:512])
                nc.scalar.dma_start(msk_s[:], msk_d)
                nc.sync.dma_start(vp_s[:, 4 * VW : 8 * VW], vp_d[:, 4 * VW : 8 * VW])
                nc.scalar.dma_start(qt_s[:, SUPER:QLOC], qt_d[:, SUPER:QLOC])
                for i in range(1, NSUP):
                    g0, nb = _G0S[i], SLOTS[i] // 128
                    c0, c1 = (g0 // 2) * 128, ((g0 + nb) // 2) * 128
                    nc.sync.dma_start(kt_s[:, c0:c1], kt_d[:, c0:c1])
                    eng = nc.scalar if i % 2 else nc.sync
                    eng.dma_start(
                        vp_s[:, g0 * VW : (g0 + nb) * VW],
                        vp_d[:, g0 * VW : (g0 + nb) * VW],
                    )
                held.update(kt_s=kt_s, vp_s=vp_s, qt_s=qt_s)

            if part == "compute":
                loads()  # once, outside the timed loop

            def body():
                if part != "compute":
                    loads()
                kt_s, vp_s, qt_s = held["kt_s"], held["vp_s"], held["qt_s"]
                if part == "dma":
                    return

                # flat software-pipelined schedule: emit score(u) + exp(u),
                # then PV(u-LAG) — keeps the in-order PE queue busy with the
                # next unit's score matmuls while ACT/DVE compute exp(u)
                units = []  # (kind, i, j, nt)
                for i in range(NSUP):
                    nb = SLOTS[i] // 128
                    diags = [("diag", i, j, 1) for j in range(4)]
                    fulls_u = [
                        ("full", i, j, min(GRP, nb - j))
                        for j in range(4, nb, GRP)
                    ]
                    if ILV and i < NSUP - 1:
                        # spread the 4 diag units between full groups so the
                        # ACT-diag -> DVE-mul chain overlaps full-group exps
                        merged, fi = [], 0
                        for d in diags:
                            merged.append(d)
                            take = max(1, len(fulls_u) // 4) if fulls_u else 0
                            merged.extend(fulls_u[fi : fi + take])
                            fi += take
                        merged.extend(fulls_u[fi:])
                        units.extend(merged)
                    elif ILV:
                        # last super: diags at the end -> short tail exp
                        units.extend(fulls_u + diags)
                    else:
                        units.extend(diags + fulls_u)

                oaccs = {}
                pexs = {}
                done_supers = []
                last_unit = {}
                first_unit = {}
                for u in units:
                    if u[1] not in first_unit:
                        first_unit[u[1]] = u
                    last_unit[u[1]] = u

                def emit_front(u):
                    kind, i, j, nt = u
                    g0, nb = _G0S[i], SLOTS[i] // 128
                    if kind == "diag":
                        g = g0 + j
                        fre = SUPER - 128 * j
                        h = (g % 2) * 64
                        psd = ps_pool.tile([128, PSW], f32, tag="ps", name="psd")
                        nc.tensor.matmul(
                            psd[:, 0:fre],
                            fr(kt_s[h : h + 64, (g // 2) * 128 : (g // 2) * 128 + 128]),
                            fr(qt_s[h : h + 64, i * SUPER + 128 * j : (i + 1) * SUPER]),
                            start=True,
                            stop=True,
                        )
                        pexd = pe_pool.tile([128, PSW], bf16, tag="pexp", name="pexd")
                        nc.scalar.activation(
                            pexd[:, 0:fre], psd[:, 0:fre], Exp, scale=ASCALE
                        )
                        if TRIENG == "gpsimd":
                            nc.gpsimd.tensor_mul(
                                pexd[:, 0:128], pexd[:, 0:128], tri_s[:]
                            )
                        else:
                            nc.vector.tensor_mul(
                                pexd[:, 0:128], pexd[:, 0:128], tri_s[:]
                            )
                        pexs[u] = pexd
                    else:
                        ps = ps_pool.tile([128, PSW], f32, tag="ps", name="ps")
                        for t in range(nt):
                            g = g0 + j + t
                            h = (g % 2) * 64
                            nc.tensor.matmul(
                                ps[:, t * 512 : (t + 1) * 512],
                                fr(kt_s[h : h + 64, (g // 2) * 128 : (g // 2) * 128 + 128]),
                                fr(qt_s[h : h + 64, i * SUPER : (i + 1) * SUPER]),
                                start=True,
                                stop=True,
                            )
                        if sched[(i, j)] == "dve":
                            pex = pi_pool.tile([128, PSW], bf16, tag="pexpi", name="pexpi")
                            nc.vector._custom_dve(
                                exp2_op,
                                out=pex[:, 0 : nt * 512].bitcast(i16),
                                in0=ps[:, 0 : nt * 512],
                                in1=msk_s[:],
                                s0=C0V,
                                s1=C1V,
                                imm2=C2V,
                            )
                        else:
                            pex = pe_pool.tile([128, PSW], bf16, tag="pexp", name="pexp")
                            nc.scalar.activation(
                                pex[:, 0 : nt * 512], ps[:, 0 : nt * 512], Exp,
                                scale=ASCALE,
                            )
                        pexs[u] = pex

                def emit_pv(u):
                    kind, i, j, nt = u
                    g0, nb = _G0S[i], SLOTS[i] // 128
                    if i not in oaccs:
                        oaccs[i] = oacc_pool.tile(
                            [VW, SUPER], f32, tag="oacc", name="oacc"
                        )
                    oacc = oaccs[i]
                    pex = pexs.pop(u)
                    if kind == "diag":
                        g = g0 + j
                        fre = SUPER - 128 * j
                        nc.tensor.matmul(
                            oacc[:, 128 * j : SUPER],
                            vp_s[:, g * VW : (g + 1) * VW],
                            pex[:, 0:fre],
                            start=(u == first_unit[i]),
                            stop=(u == last_unit[i]),
                            skip_group_check=True,
                        )
                    else:
                        for t in range(nt):
                            g = g0 + j + t
                            nc.tensor.matmul(
                                oacc[:, 0:SUPER],
                                vp_s[:, g * VW : (g + 1) * VW],
                                pex[:, t * 512 : (t + 1) * 512],
                                start=(u == first_unit[i] and t == 0),
                                stop=(u == last_unit[i] and t == nt - 1),
                                skip_group_check=True,
                            )
                    if u == last_unit[i]:
                        done_supers.append([i, oacc, None])
                        del oaccs[i]

                def emit_tail():
                    # super finished: O^T to DRAM via SBUF bounce (DMA can't
                    # read PSUM); host transposes+divides.  Emitted TLAG units
                    # late so queued exps run before the copy's PV-wait.
                    i, oacc, _ = done_supers.pop(0)
                    ocp = ocp_pool.tile([VW, SUPER], f32, tag="ocp", name="ocp")
                    if os.environ.get("KERNEL_OCP", "vector") == "vector":
                        nc.vector.tensor_copy(ocp[:], oacc[:])
                    else:
                        nc.scalar.copy(ocp[:], oacc[:])
                    nc.sync.dma_start(o_d[:, i * SUPER : (i + 1) * SUPER], ocp[:])

                LAG = int(os.environ.get("KERNEL_LAG", "1"))
                TLAG = int(os.environ.get("KERNEL_TLAG", "2"))
                for idx, u in enumerate(units):
                    emit_front(u)
                    if idx >= LAG:
                        emit_pv(units[idx - LAG])
                        for d in done_supers:
                            if d[2] is None:
                                d[2] = idx
                    while done_supers and idx >= done_supers[0][2] + TLAG:
                        emit_tail()
                for u in units[len(units) - LAG :]:
                    emit_pv(u)
                while done_supers:
                    emit_tail()

            if reps > 1:
                with tc.For_i(
                    0, reps, 1, hint_engines=(mybir.EngineType.PE,)
                ) as _:
                    body()
            else:
                body()

    nc.compile()
    return nc


def _prep_core_inputs(q, k, v, b, a, dve_blk):
    """Host-side layout prep for one core (pure data movement, no flops)."""
    q0s = [SUPER * (2 * i + a) for i in range(NSUP)]
    # local Q rows (super-major) -> Q^T duplicated on both partition halves,
    # pre-scaled so PSUM scores arrive as logit*log2e*2^23
    qs = np.concatenate([q[b, q0 : q0 + SUPER] for q0 in q0s], axis=0)  # [2048, 64]
    qs = (qs.astype(np.float64) * QSCALE).astype(np.float32)
    qt = np.concatenate([qs.T, qs.T], axis=0)  # [128, 2048]

    k_parts, v_parts = [], []
    for i, q0 in enumerate(q0s):
        slot = SLOTS[i]
        pad = slot - SUPER - q0
        kp = [k[b, q0 : q0 + SUPER], k[b, 0:q0]]
        va = np.concatenate(
            [v[b, 0 : q0 + SUPER], np.ones((q0 + SUPER, 1), np.float32)], axis=1
        )
        vvp = [va[q0 : q0 + SUPER], va[0:q0]]
        if pad:
            kp.append(np.zeros((pad, D), np.float32))
            vvp.append(np.zeros((pad, VW), np.float32))
        k_parts.append(np.concatenate(kp, axis=0))
        v_parts.append(np.concatenate(vvp, axis=0))
    k_arr = np.concatenate(k_parts, axis=0)  # [10240, 64]
    v_arr = np.concatenate(v_parts, axis=0)  # [10240, 65]

    # kt: block g -> partitions (g%2)*64..+64, columns (g//2)*128..+128
    kt = np.zeros((128, GTOT // 2 * 128), np.float32)
    kb = k_arr.reshape(GTOT, 128, D)
    for g in range(GTOT):
        h = (g % 2) * 64
        kt[h : h + 64, (g // 2) * 128 : (g // 2) * 128 + 128] = kb[g].T
    # vp: partition-major [p, g*65:(g+1)*65] = v_arr[g*128+p]; DVE-path blocks
    # carry the 2^-64 compensation for the custom-op's exponent bias
    import ml_dtypes

    vb = v_arr.reshape(GTOT, 128, VW).astype(np.float64)
    for g in dve_blk:
        vb[g] *= VSCALE
    vp = np.ascontiguousarray(
        vb.transpose(1, 0, 2).reshape(128, GTOT * VW).astype(ml_dtypes.bfloat16)
    )
    return {"qt": np.ascontiguousarray(qt), "kt": kt, "vp": vp}


def make_in_maps(q, k, v):
    import ml_dtypes

    tri = np.triu(np.ones((128, 128), ml_dtypes.bfloat16))  # valid: k_row <= q_col
    msk = np.full((128, 1), 0x0000FFFF, np.uint32).view(np.float32)
    dve_blk = _dve_blocks(_schedule())
    in_maps = []
    for c in range(NCORES):
        m = _prep_core_inputs(q, k, v, c // 2, c % 2, dve_blk)
        m["tri"] = tri
        m["msk"] = msk
        in_maps.append(m)
    return in_maps


def kernel(q, k, v):
    from concourse.bass_utils import run_bass_kernel_spmd

    q = np.asarray(q, np.float32)
    k = np.asarray(k, np.float32)
    v = np.asarray(v, np.float32)

    if "nc" not in _CACHE:
        _CACHE["nc"] = _build_nc()
    nc = _CACHE["nc"]

    in_maps = make_in_maps(q, k, v)
    res = run_bass_kernel_spmd(
        nc,
        in_maps,
        core_ids=list(range(NCORES)),
        trace=bool(int(os.environ.get("KERNEL_TRACE", "0"))),
    )
    _CACHE["last_result"] = res

    out = np.empty((B, S, D), np.float32)
    for c in range(NCORES):
        b, a = c // 2, c % 2
        ot = res.results[c]["o"]  # [65, 2048] = O^T with sums in row 64
        for i in range(NSUP):
            q0 = SUPER * (2 * i + a)
            blk = ot[:, i * SUPER : (i + 1) * SUPER].astype(np.float64)
            out[b, q0 : q0 + SUPER] = (blk[0:D] / blk[D : D + 1]).T
    return out



# revision 4
# speedup vs baseline: 1.3685x; 1.3685x over previous
"""Causal attention kernel for Trainium2 (Bass/Tile), 8 NeuronCores. v3.

Problem: B=4, S=4096, D=64 fp32 causal softmax attention.

Sharding: data-parallel over batch (4 batches x 2 cores each); within a
batch the S axis of Q is split causally-balanced: core parity a takes
q-supers (512 rows) at q0 = 512*(2i+a), i=0..3.  Every core runs the SAME
static program over k-context "slots" [1024, 2048, 3072, 4096].

v3 changes vs v2:
  * K/V block DEDUPE: instead of materializing each super's full context
    (80 blocks incl. zero padding), the address space holds 56 unique
    128-row blocks: 24 shared context blocks P_0..P_23 (= K rows
    0..3072, read by every super whose context covers them), 4x4
    per-super tail blocks E_i (real context tail on odd-parity cores,
    zero padding on even-parity cores), and 4x4 per-super diagonal
    blocks D_i.  Addresses are laid out in first-use order so the DMA
    stream is a handful of large contiguous chunks arriving just in
    time.
  * Q^T and K^T are bf16 (moving/stationary matmul operands), halving
    their DMA bytes; PSUM scores still accumulate fp32.
  * both exp paths emit 2^64*exp(t): the DVE bit-pattern op does so
    inherently; the ScalarE ACT path now adds bias=64*ln2.  The factor
    cancels in the host's numerator/denominator divide, so V needs no
    per-path pre-scaling and any engine may consume any block.
  * O^T is stored/DMA'd as bf16 (host divides in float64).

Math layout (per 512-row q-super, context slot L): scores^T[k,q] = K @ Q^T
per 128-row k-block (contraction D=64 on partitions; even/odd addresses on
partition halves 0-63/64-127 so adjacent matmuls hit disjoint PE row-groups
and overlap).  P^T tiles feed O^T[d,q] += matmul with V_aug [128, 65] as
stationary (col 64 = ones -> row sums in O^T row 64).  Diagonal k-blocks are
free-dim-trimmed, ScalarE-exp'd and triangle-masked with a host triu mask.

KERNEL_REPS (build-time env, default 1): wraps the body in a hardware
For_i loop for benchmarking; the shipped/default program has no loop.
"""

import os
import numpy as np

B, S, D = 4, 4096, 64
NCORES = 8
SUPER = 512  # q rows per super-block
SLOTS = [1024, 2048, 3072, 4096]  # k-context slot per super index
NSUP = 4  # supers per core
QLOC = NSUP * SUPER  # 2048 local q rows
VW = D + 1  # v row width with ones column

# deduped address space: 56 blocks in first-use order
# [D_0 | E_0 | D_1 | P_0..7 | E_1 | D_2 | P_8..15 | E_2 | D_3 | P_16..23 | E_3]
DBASE = [0, 8, 24, 40]
EBASE = [4, 20, 36, 52]
NADDR = 56


def _paddr(m):
    return 12 + m if m < 8 else (28 + m - 8 if m < 16 else 44 + m - 16)


def _g_addr(i, j):
    """Slot-local block j (0..SLOTS[i]/128-1) of super i -> address."""
    if j < 4:
        return DBASE[i] + j  # diagonal
    if j < 8 * i + 4:
        return _paddr(j - 4)  # shared context prefix
    return EBASE[i] + (j - 8 * i - 4)  # per-super tail (real or zero-pad)


def _content_block(a, addr):
    """Address -> 128-row block index of K (None = zeros), for parity a."""
    for i in range(NSUP):
        if DBASE[i] <= addr < DBASE[i] + 4:
            return 4 * (2 * i + a) + (addr - DBASE[i])
        if EBASE[i] <= addr < EBASE[i] + 4:
            return 8 * i + (addr - EBASE[i]) if a == 1 else None
    for m in range(24):
        if _paddr(m) == addr:
            return m
    raise AssertionError(addr)


LOG2E = 1.4426950408889634
QSCALE = LOG2E * (1 << 7) / np.sqrt(D)  # host premultiplies Q^T by this
ASCALE = float(np.log(2.0) / (1 << 7))  # ScalarE ACT scale immediate
BEXP = float(64 * np.log(2.0))  # ACT bias: emit 2^64*exp to match DVE path
# minimax quadratic for h(f) = 2^f - 1 - f on [0,1]
HA, HB, HC = 0.34399492, -0.34894542, 0.00247407
C0V = float(np.float32((191 + HC) * (1 << 7)))  # anchor + poly const + exp bias
C1V = float(np.float32(HA * 2.0**21))
C2V = float(np.float32(HB * 2.0**14))

GRP = int(os.environ.get("KERNEL_GROUP", "1"))
# target DVE-path full-block columns per core (engine balance knob)
DVE_COLS = int(os.environ.get("KERNEL_DVE_COLS", "15872"))
USE_DVE = os.environ.get("KERNEL_DVE", "1") == "1"

_CACHE = {}


def _schedule():
    """Assign each full-block group to 'dve' or 'act', spread evenly."""
    fulls = []
    for i in range(NSUP):
        nb = SLOTS[i] // 128
        for j in range(4, nb, GRP):
            nt = min(GRP, nb - j)
            fulls.append((i, j, nt))
    sched = {}
    if not USE_DVE:
        return {(i, j): "act" for i, j, _ in fulls}
    total = sum(nt * 512 for _, _, nt in fulls)
    frac = min(1.0, DVE_COLS / total)
    dve = run = 0.0
    for i, j, nt in fulls:
        cols = nt * 512
        run += cols
        if dve + cols <= frac * run + 1e-9:
            sched[(i, j)] = "dve"
            dve += cols
        else:
            sched[(i, j)] = "act"
    return sched


def _register_exp2():
    """Register the custom fused DVE exp op (idempotent)."""
    from concourse import dve_ops
    from concourse.dve_spec import (
        Spec, Src0, C0, C1, C2, C3, One, Bin, AluOp, lower,
        _spill_c3_to_src1,
    )
    from concourse.dve_uop import DveOpSpec

    name = "EXP2_SCHR_ANT"
    for op in dve_ops.OPS:
        if op.name == name:
            return op

    # the 0xFFFF frac mask rides C3 (latched from in1 at element 0 — the
    # streaming [P,1] Src1 broadcast hangs the DVE on this runtime)
    a = Src0 + C0
    fb = Bin(AluOp.BITWISE_AND, a, C3)
    fo = Bin(AluOp.BITWISE_OR, fb, One)
    fm = fo - One
    body = _spill_c3_to_src1(a + (fm * C1 + C2) * fm)

    def ref(in0, in1, s0, s1, imm2):
        av = (np.asarray(in0, np.float32) + np.float32(s0)).astype(np.float32)
        m = np.broadcast_to(np.asarray(in1, np.float32), av.shape)
        fb_ = av.view(np.uint32) & m.view(np.uint32)
        fo_ = fb_ | np.float32(1.0).view(np.uint32)
        fm_ = (fo_.view(np.float32) - np.float32(1.0)).astype(np.float32)
        p3 = ((fm_ * np.float32(s1) + np.float32(imm2)) * fm_).astype(np.float32)
        return (av + p3).astype(np.float32)

    spec = Spec(body=body, reference=ref)
    row = dve_ops._CUSTOM_DVE_ROW_BASE + len(dve_ops.OPS)
    shas = {
        ver: DveOpSpec(name=name, opcode=row, uops=lower(spec, ver=ver),
                       rd1_en=True).sha(ver)
        for ver in ("v3", "v4")
    }
    op = dve_ops.DveOp(name, spec, subdim=False, uops_sha=shas)
    dve_ops.OPS.append(op)
    dve_ops._SUB_OPCODE_FOR_NAME[name] = row
    dve_ops.CUSTOM_DVE_SPECS[name] = spec
    return op


def _build_nc(reps=None):
    import concourse.bass as bass
    import concourse.tile as tile
    import concourse.mybir as mybir
    from concourse import bacc

    if reps is None:
        reps = int(os.environ.get("KERNEL_REPS", "1"))
    f32 = mybir.dt.float32
    i16 = mybir.dt.int16
    bf16 = mybir.dt.bfloat16
    nc = bacc.Bacc("TRN2", target_bir_lowering=False, debug=False, num_devices=NCORES)

    # Q^T/K^T matmul operand dtype: bf16 (default) or float32r fallback
    qkdt = os.environ.get("KERNEL_QKDT", "bf16")
    fdt = bf16 if qkdt == "bf16" else mybir.dt.float32r
    fr = lambda ap: ap

    exp2_op = _register_exp2() if USE_DVE else None
    sched = _schedule()

    qt_d = nc.dram_tensor("qt", [128, QLOC], fdt, kind="ExternalInput").ap()
    kt_d = nc.dram_tensor("kt", [128, NADDR // 2 * 128], fdt, kind="ExternalInput").ap()
    vp_d = nc.dram_tensor("vp", [128, NADDR * VW], bf16, kind="ExternalInput").ap()
    aux_d = nc.dram_tensor("aux", [128, 66], f32, kind="ExternalInput").ap()
    o_d = nc.dram_tensor("o", [VW, QLOC], bf16, kind="ExternalOutput").ap()

    Exp = mybir.ActivationFunctionType.Exp
    TRIENG = os.environ.get("KERNEL_TRIENG", "vector")
    ILV = os.environ.get("KERNEL_ILV", "1") == "1"

    PSW = 512 * GRP
    with tile.TileContext(nc) as tc:
        with (
            tc.tile_pool(name="inp", bufs=int(os.environ.get("KERNEL_INPBUFS", "2"))) as inp,
            tc.tile_pool(name="pexp_pool", bufs=4) as pe_pool,
            tc.tile_pool(name="pexpi_pool", bufs=4) as pi_pool,
            tc.tile_pool(name="ocp_pool", bufs=2) as ocp_pool,
            tc.tile_pool(name="ps_pool", bufs=max(2, (8 - 2) // GRP), space="PSUM") as ps_pool,
            tc.tile_pool(name="oacc_pool", bufs=2, space="PSUM") as oacc_pool,
        ):
            aux_s = inp.tile([128, 66], f32)
            msk_s = aux_s[:, 0:1]
            bex_s = aux_s[:, 1:2]
            tri_s = aux_s[:, 2:66].bitcast(bf16)

            part = os.environ.get("KERNEL_PART", "all")  # all|dma|compute
            held = {}

            def loads():
                kt_s = inp.tile([128, NADDR // 2 * 128], fdt, tag="kt_s", name="kt_s")
                vp_s = inp.tile([128, NADDR * VW], bf16, tag="vp_s", name="vp_s")
                qt_s = inp.tile([128, QLOC], fdt, tag="qt_s", name="qt_s")
                # address space is in first-use order; stream it as a few
                # large chunks on both HWDGE rings, earliest-needed first.
                # chunk boundaries (in addresses): super 0 = [0,8), then
                # [8,24), [24,40), [40,56) cover supers 1-3 just in time.
                def ktc(a0, a1):
                    return slice((a0 // 2) * 128, (a1 // 2) * 128)

                def vpc(a0, a1):
                    return slice(a0 * VW, a1 * VW)

                nc.sync.dma_start(kt_s[:, ktc(0, 4)], kt_d[:, ktc(0, 4)])
                nc.scalar.dma_start(aux_s[:], aux_d)
                nc.sync.dma_start(kt_s[:, ktc(4, 8)], kt_d[:, ktc(4, 8)])
                nc.scalar.dma_start(qt_s[:, 0:SUPER], qt_d[:, 0:SUPER])
                nc.sync.dma_start(kt_s[:, ktc(8, 24)], kt_d[:, ktc(8, 24)])
                nc.scalar.dma_start(vp_s[:, vpc(0, 8)], vp_d[:, vpc(0, 8)])
                nc.sync.dma_start(kt_s[:, ktc(24, 56)], kt_d[:, ktc(24, 56)])
                nc.scalar.dma_start(qt_s[:, SUPER:QLOC], qt_d[:, SUPER:QLOC])
                nc.scalar.dma_start(vp_s[:, vpc(8, 24)], vp_d[:, vpc(8, 24)])
                nc.sync.dma_start(vp_s[:, vpc(24, 56)], vp_d[:, vpc(24, 56)])
                held.update(kt_s=kt_s, vp_s=vp_s, qt_s=qt_s)

            if part == "compute":
                loads()  # once, outside the timed loop

            def body():
                if part != "compute":
                    loads()
                kt_s, vp_s, qt_s = held["kt_s"], held["vp_s"], held["qt_s"]
                if part == "dma":
                    return

                # flat software-pipelined schedule: emit score(u) + exp(u),
                # then PV(u-LAG) — keeps the in-order PE queue busy with the
                # next unit's score matmuls while ACT/DVE compute exp(u)
                units = []  # (kind, i, j, nt)
                for i in range(NSUP):
                    nb = SLOTS[i] // 128
                    diags = [("diag", i, j, 1) for j in range(4)]
                    fulls_u = [
                        ("full", i, j, min(GRP, nb - j))
                        for j in range(4, nb, GRP)
                    ]
                    if ILV and i < NSUP - 1:
                        # spread the 4 diag units between full groups so the
                        # ACT-diag -> DVE-mul chain overlaps full-group exps
                        merged, fi = [], 0
                        for d in diags:
                            merged.append(d)
                            take = max(1, len(fulls_u) // 4) if fulls_u else 0
                            merged.extend(fulls_u[fi : fi + take])
                            fi += take
                        merged.extend(fulls_u[fi:])
                        units.extend(merged)
                    elif ILV:
                        # last super: diags at the end -> short tail exp
                        units.extend(fulls_u + diags)
                    else:
                        units.extend(diags + fulls_u)

                oaccs = {}
                pexs = {}
                done_supers = []
                last_unit = {}
                first_unit = {}
                for u in units:
                    if u[1] not in first_unit:
                        first_unit[u[1]] = u
                    last_unit[u[1]] = u

                def emit_front(u):
                    kind, i, j, nt = u
                    if kind == "diag":
                        g = _g_addr(i, j)
                        fre = SUPER - 128 * j
                        h = (g % 2) * 64
                        psd = ps_pool.tile([128, PSW], f32, tag="ps", name="psd")
                        nc.tensor.matmul(
                            psd[:, 0:fre],
                            fr(kt_s[h : h + 64, (g // 2) * 128 : (g // 2) * 128 + 128]),
                            fr(qt_s[h : h + 64, i * SUPER + 128 * j : (i + 1) * SUPER]),
                            start=True,
                            stop=True,
                        )
                        pexd = pe_pool.tile([128, PSW], bf16, tag="pexp", name="pexd")
                        nc.scalar.activation(
                            pexd[:, 0:fre], psd[:, 0:fre], Exp,
                            bias=bex_s, scale=ASCALE,
                        )
                        if TRIENG == "gpsimd":
                            nc.gpsimd.tensor_mul(
                                pexd[:, 0:128], pexd[:, 0:128], tri_s
                            )
                        else:
                            nc.vector.tensor_mul(
                                pexd[:, 0:128], pexd[:, 0:128], tri_s
                            )
                        pexs[u] = pexd
                    else:
                        ps = ps_pool.tile([128, PSW], f32, tag="ps", name="ps")
                        for t in range(nt):
                            g = _g_addr(i, j + t)
                            h = (g % 2) * 64
                            nc.tensor.matmul(
                                ps[:, t * 512 : (t + 1) * 512],
                                fr(kt_s[h : h + 64, (g // 2) * 128 : (g // 2) * 128 + 128]),
                                fr(qt_s[h : h + 64, i * SUPER : (i + 1) * SUPER]),
                                start=True,
                                stop=True,
                            )
                        if sched[(i, j)] == "dve":
                            pex = pi_pool.tile([128, PSW], bf16, tag="pexpi", name="pexpi")
                            nc.vector._custom_dve(
                                exp2_op,
                                out=pex[:, 0 : nt * 512].bitcast(i16),
                                in0=ps[:, 0 : nt * 512],
                                in1=msk_s,
                                s0=C0V,
                                s1=C1V,
                                imm2=C2V,
                            )
                        else:
                            pex = pe_pool.tile([128, PSW], bf16, tag="pexp", name="pexp")
                            nc.scalar.activation(
                                pex[:, 0 : nt * 512], ps[:, 0 : nt * 512], Exp,
                                bias=bex_s, scale=ASCALE,
                            )
                        pexs[u] = pex

                def emit_pv(u):
                    kind, i, j, nt = u
                    if i not in oaccs:
                        oaccs[i] = oacc_pool.tile(
                            [VW, SUPER], f32, tag="oacc", name="oacc"
                        )
                    oacc = oaccs[i]
                    pex = pexs.pop(u)
                    if kind == "diag":
                        g = _g_addr(i, j)
                        fre = SUPER - 128 * j
                        nc.tensor.matmul(
                            oacc[:, 128 * j : SUPER],
                            vp_s[:, g * VW : (g + 1) * VW],
                            pex[:, 0:fre],
                            start=(u == first_unit[i]),
                            stop=(u == last_unit[i]),
                            skip_group_check=True,
                        )
                    else:
                        for t in range(nt):
                            g = _g_addr(i, j + t)
                            nc.tensor.matmul(
                                oacc[:, 0:SUPER],
                                vp_s[:, g * VW : (g + 1) * VW],
                                pex[:, t * 512 : (t + 1) * 512],
                                start=(u == first_unit[i] and t == 0),
                                stop=(u == last_unit[i] and t == nt - 1),
                                skip_group_check=True,
                            )
                    if u == last_unit[i]:
                        done_supers.append([i, oacc, None])
                        del oaccs[i]

                def emit_tail():
                    # super finished: O^T to DRAM via SBUF bounce (DMA can't
                    # read PSUM); host transposes+divides.  Emitted TLAG units
                    # late so queued exps run before the copy's PV-wait.
                    i, oacc, _ = done_supers.pop(0)
                    ocp = ocp_pool.tile([VW, SUPER], bf16, tag="ocp", name="ocp")
                    if os.environ.get("KERNEL_OCP", "vector") == "vector":
                        nc.vector.tensor_copy(ocp[:], oacc[:])
                    else:
                        nc.scalar.copy(ocp[:], oacc[:])
                    nc.sync.dma_start(o_d[:, i * SUPER : (i + 1) * SUPER], ocp[:])

                LAG = int(os.environ.get("KERNEL_LAG", "3"))
                TLAG = int(os.environ.get("KERNEL_TLAG", "2"))
                for idx, u in enumerate(units):
                    emit_front(u)
                    if idx >= LAG:
                        emit_pv(units[idx - LAG])
                        for d in done_supers:
                            if d[2] is None:
                                d[2] = idx
                    while done_supers and idx >= done_supers[0][2] + TLAG:
                        emit_tail()
                for u in units[len(units) - LAG :]:
                    emit_pv(u)
                while done_supers:
                    emit_tail()

            if reps > 1:
                with tc.For_i(
                    0, reps, 1, hint_engines=(mybir.EngineType.PE,)
                ) as _:
                    body()
            else:
                body()

    nc.compile()
    return nc


def _prep_core_inputs(q, k, v, b, a):
    """Host-side layout prep for one core (pure data movement, no flops)."""
    import ml_dtypes

    bf = ml_dtypes.bfloat16
    q0s = [SUPER * (2 * i + a) for i in range(NSUP)]
    # local Q rows (super-major) -> Q^T duplicated on both partition halves,
    # pre-scaled so PSUM scores arrive as logit*log2e*2^7
    qs = np.concatenate([q[b, q0 : q0 + SUPER] for q0 in q0s], axis=0)  # [2048, 64]
    qs = (qs.astype(np.float64) * QSCALE).astype(bf)
    qt = np.concatenate([qs.T, qs.T], axis=0)  # [128, 2048]

    kb = k[b].reshape(S // 128, 128, D)  # 32 K blocks
    va = np.concatenate([v[b], np.ones((S, 1), np.float32)], axis=1)
    vb = va.reshape(S // 128, 128, VW)

    # kt: addr g -> partitions (g%2)*64..+64, columns (g//2)*128..+128
    kt = np.zeros((128, NADDR // 2 * 128), bf)
    vpb = np.zeros((NADDR, 128, VW), np.float32)
    for g in range(NADDR):
        c = _content_block(a, g)
        if c is None:
            continue
        h = (g % 2) * 64
        kt[h : h + 64, (g // 2) * 128 : (g // 2) * 128 + 128] = kb[c].T.astype(bf)
        vpb[g] = vb[c]
    # vp: partition-major [p, g*65:(g+1)*65] = block g row p
    vp = np.ascontiguousarray(
        vpb.transpose(1, 0, 2).reshape(128, NADDR * VW).astype(bf)
    )
    return {"qt": np.ascontiguousarray(qt), "kt": kt, "vp": vp}


def make_in_maps(q, k, v):
    import ml_dtypes

    tri = np.triu(np.ones((128, 128), ml_dtypes.bfloat16))  # valid: k_row <= q_col
    aux = np.empty((128, 66), np.float32)
    aux[:, 0:1] = np.full((128, 1), 0x0000FFFF, np.uint32).view(np.float32)
    aux[:, 1:2] = BEXP
    aux[:, 2:66] = tri.view(np.uint16).view(np.float32)
    in_maps = []
    for c in range(NCORES):
        m = _prep_core_inputs(q, k, v, c // 2, c % 2)
        m["aux"] = aux
        in_maps.append(m)
    return in_maps


def kernel(q, k, v):
    from concourse.bass_utils import run_bass_kernel_spmd

    q = np.asarray(q, np.float32)
    k = np.asarray(k, np.float32)
    v = np.asarray(v, np.float32)

    if "nc" not in _CACHE:
        _CACHE["nc"] = _build_nc()
    nc = _CACHE["nc"]

    in_maps = make_in_maps(q, k, v)
    res = run_bass_kernel_spmd(
        nc,
        in_maps,
        core_ids=list(range(NCORES)),
        trace=bool(int(os.environ.get("KERNEL_TRACE", "0"))),
    )
    _CACHE["last_result"] = res

    out = np.empty((B, S, D), np.float32)
    for c in range(NCORES):
        b, a = c // 2, c % 2
        ot = res.results[c]["o"]  # [65, 2048] = O^T with sums in row 64
        for i in range(NSUP):
            q0 = SUPER * (2 * i + a)
            blk = ot[:, i * SUPER : (i + 1) * SUPER].astype(np.float64)
            out[b, q0 : q0 + SUPER] = (blk[0:D] / blk[D : D + 1]).T
    return out
